# revision 25
# baseline (speedup 1.0000x reference)
"""Trainium2 Bass kernel for nn_MultiHeadAttention (B=2, T=2048, D=1024, H=16).

Sharding: 8 cores; core c owns head pair (2c, 2c+1) = output-channel slice
[c*128, (c+1)*128) of Wq/Wk/Wv columns and Wo rows (tensor parallel), both
batches. Host pre-transposes x and weight slices (cast to f16); each core
computes a partial output projection over its 128 ctx channels in f16; host
sums the 8 partials in f32 (replaces the all-reduce) and adds bo.

The kernel is one dense exp stream on the scalar engine (the binding engine
at ~1.0us per [128,1024] score tile, 128 tiles) with all other work
(projections, V transposes, out-projection, softmax normalization) scheduled
into per-ktile drain slots under it. The 8 attention chunks form a single
flat 128-slot pipeline: scores(kt) -> exp(kt) -> ctx(kt) with scores one
slot ahead of ctx, continuing seamlessly across chunk boundaries; the
previous chunk's finalize (reciprocal of the softmax denominators staged to
partition 0 via a small DMA, PE broadcast, normalize, out-projection) is
drained through the following chunk's slots.
"""

import numpy as np

P = 128
D = 1024
BT = 4096
T = 2048
NB = 2
DC = 8    # D chunks of 128
KT = 16   # 128-wide k-tiles per batch
NCORES = 8
DK = 64

_CACHE = {}


def _build(reps=1, debug=False):
    import concourse.bass as bass
    import concourse.tile as tile
    from concourse import bacc, mybir
    from concourse.masks import make_identity

    f32 = mybir.dt.float32
    f16 = mybir.dt.float16
    f8 = mybir.dt.float8e4
    DR = mybir.MatmulPerfMode.DoubleRow
    Exp = mybir.ActivationFunctionType.Exp
    ds = bass.ds

    nc = bacc.Bacc("TRN2", target_bir_lowering=False, debug=False)

    xt = nc.dram_tensor("xt", [D, BT], f16, kind="ExternalInput").ap()
    wq = nc.dram_tensor("wq", [D, P], f16, kind="ExternalInput").ap()
    wk = nc.dram_tensor("wk", [D, P], f16, kind="ExternalInput").ap()
    wv = nc.dram_tensor("wv", [D, P], f16, kind="ExternalInput").ap()
    wo = nc.dram_tensor("wo", [P, D], f16, kind="ExternalInput").ap()
    bqd = nc.dram_tensor("bq", [P, 1], f32, kind="ExternalInput").ap()
    bkd = nc.dram_tensor("bk", [P, 1], f32, kind="ExternalInput").ap()
    bvd = nc.dram_tensor("bv", [P, 1], f32, kind="ExternalInput").ap()
    out = nc.dram_tensor("out", [BT, D], f16, kind="ExternalOutput").ap()
    dbg = {}
    if debug:
        dbg["qt"] = nc.dram_tensor("dbg_qt", [P, T], f16, kind="ExternalOutput").ap()
        dbg["kt"] = nc.dram_tensor("dbg_kt", [P, T], f16, kind="ExternalOutput").ap()
        dbg["va"] = nc.dram_tensor("dbg_va", [P, KT * 65], f16, kind="ExternalOutput").ap()
        dbg["vb"] = nc.dram_tensor("dbg_vb", [P, KT * 65], f16, kind="ExternalOutput").ap()
        dbg["esc"] = nc.dram_tensor("dbg_esc", [P, 1024], f16, kind="ExternalOutput").ap()
        dbg["ua"] = nc.dram_tensor("dbg_ua", [65, 512], f32, kind="ExternalOutput").ap()
        dbg["ub"] = nc.dram_tensor("dbg_ub", [65, 512], f32, kind="ExternalOutput").ap()
        dbg["rr"] = nc.dram_tensor("dbg_rr", [1, 1024], f16, kind="ExternalOutput").ap()
        dbg["ctq"] = nc.dram_tensor("dbg_ctq", [P, 512], f16, kind="ExternalOutput").ap()

    with tile.TileContext(nc) as tc:
        with (
            tc.tile_pool(name="const", bufs=1) as constp,
            tc.tile_pool(name="xtp", bufs=3) as xtp,
            tc.tile_pool(name="qkv", bufs=1) as qkvp,
            tc.tile_pool(name="vts", bufs=2) as vtsp,
            tc.tile_pool(name="esc", bufs=3) as escp,
            tc.tile_pool(name="ctq", bufs=2) as ctqp,
            tc.tile_pool(name="small", bufs=2) as smallp,
            tc.tile_pool(name="bsb", bufs=2) as bsbp,
            tc.tile_pool(name="posb", bufs=3) as posbp,
            # PSUM: sc 2x2 banks + cx 2x1 banks + flex 2x1 banks = 8
            tc.tile_pool(name="psS", bufs=2, space="PSUM") as psS,
            tc.tile_pool(name="psC", bufs=2, space="PSUM") as psC,
            tc.tile_pool(name="psF", bufs=2, space="PSUM") as psF,
        ):
            # ---- constants / weights; DMA order puts wk and x(0) first so
            # the first projection can start as early as possible ----
            wq_sb = constp.tile([P, DC, P], f16, tag="wq")
            wk_sb = constp.tile([P, DC, P], f16, tag="wk")
            wv_sb = constp.tile([P, DC, P], f16, tag="wv")
            wo_sb = constp.tile([P, D], f16, tag="wo")
            bq_sb = constp.tile([P, 1], f32, tag="bq")
            bk_sb = constp.tile([P, 1], f32, tag="bk")
            bv_sb = constp.tile([P, 1], f32, tag="bv")
            ident_f = constp.tile([P, P], f32, tag="identf")
            make_identity(nc, ident_f)
            ident = constp.tile([P, P], f16, tag="ident")
            nc.vector.tensor_copy(ident, ident_f)
            ones16 = constp.tile([P, 512], f16, tag="ones16")
            nc.vector.memset(ones16, 1.0)

            xt_r = xt.rearrange("(c p) t -> p c t", p=P)
            xtiles = {}

            def load_x(tch, eng=None):
                def th():
                    t0 = xtp.tile([P, DC, 512], f16, tag="xt", name=f"x{tch}")
                    (eng or nc.gpsimd).dma_start(
                        t0, xt_r[:, :, ds(tch * 512, 512)])
                    xtiles[tch] = t0
                return th

            # Head DMA: transfers on one ring are FIFO, so the sync ring
            # carries x0 (split) then x1/x2 in priority order while the
            # gpsimd ring carries the weights; biases ride the scalar queue.
            # x3..x7 are gated behind compute by the 3-deep x pool rotation.
            x0 = xtp.tile([P, DC, 512], f16, tag="xt", name="x0")
            for qq in range(4):
                nc.sync.dma_start(x0[:, 2 * qq: 2 * qq + 2],
                                  xt_r[:, 2 * qq: 2 * qq + 2, ds(0, 512)])
            xtiles[0] = x0
            nc.gpsimd.dma_start(wk_sb, wk.rearrange("(c p) e -> p c e", p=P))
            nc.scalar.dma_start(bk_sb, bkd)
            nc.scalar.dma_start(bq_sb, bqd)
            nc.scalar.dma_start(bv_sb, bvd)
            nc.gpsimd.dma_start(wq_sb, wq.rearrange("(c p) e -> p c e", p=P))
            load_x(1, nc.sync)()
            nc.gpsimd.dma_start(wv_sb, wv.rearrange("(c p) e -> p c e", p=P))
            load_x(2, nc.sync)()

            def load_wo():
                nc.sync.dma_start(wo_sb, wo)

            # ---- per-batch persistent tiles ----
            qt_sb = [
                qkvp.tile([P, T], f16, tag=f"qt{b}", name=f"qt{b}")
                for b in range(NB)
            ]
            kt_sb = [
                qkvp.tile([P, T], f16, tag=f"kt{b}", name=f"kt{b}")
                for b in range(NB)
            ]
            # V natural per batch, 65-wide blocks per ktile: [V(64)|1]; the
            # ones column accumulates the softmax denominator for free.
            # (fp8 DoubleRow ctx was tried: ~27us faster on PE but the fp8
            # V-quantization noise on peaked attention rows sums to ~2.5-4%
            # across the 8 cores' partials -- over the tolerance.)
            va_sb = [
                qkvp.tile([P, KT * 65], f16, tag=f"va{b}", name=f"va{b}")
                for b in range(NB)
            ]
            vb_sb = [
                qkvp.tile([P, KT * 65], f16, tag=f"vb{b}", name=f"vb{b}")
                for b in range(NB)
            ]
            ones_col = ones16[:, 0:KT].rearrange("p (k one) -> p k one", one=1)
            for b in range(NB):
                nc.vector.tensor_copy(
                    va_sb[b].rearrange("p (k c) -> p k c", c=65)[:, :, 64:65],
                    ones_col,
                )
                nc.vector.tensor_copy(
                    vb_sb[b].rearrange("p (k c) -> p k c", c=65)[:, :, 64:65],
                    ones_col,
                )

            _proj_ps = {}

            def proj_half(tch, w_sb, b_sb, dst, half):
                # half 0: open PSUM accumulation, 4 contraction chunks;
                # half 1: 4 more chunks, close group, evict (bias+cast f16)
                def th():
                    if half == 0:
                        ps = psF.tile([P, 512], f32, tag="fx", name="pj")
                        _proj_ps[(tch, id(w_sb))] = ps
                        for c in range(4):
                            nc.tensor.matmul(
                                ps, w_sb[:, c], xtiles[tch][:, c],
                                start=(c == 0), stop=False,
                            )
                    else:
                        ps = _proj_ps.pop((tch, id(w_sb)))
                        for c in range(4, DC):
                            nc.tensor.matmul(
                                ps, w_sb[:, c], xtiles[tch][:, c],
                                start=False, stop=(c == DC - 1),
                            )
                        nc.vector.tensor_scalar_add(dst, ps, b_sb)
                return th

            def K(tch, half):
                b = tch // 4
                dst = kt_sb[b][:, ds((tch % 4) * 512, 512)]
                return proj_half(tch, wk_sb, bk_sb, dst, half)

            def Q(tch, half):
                b = tch // 4
                dst = qt_sb[b][:, ds((tch % 4) * 512, 512)]
                return proj_half(tch, wq_sb, bq_sb, dst, half)

            _vts = {}

            def V(tch, half):
                def th():
                    if half == 0:
                        _vts[tch] = vtsp.tile([P, 512], f32, tag="vts",
                                              name=f"v{tch}")
                    proj_half(tch, wv_sb, bv_sb, _vts[tch], half)()
                return th

            def Vt(tch, half):
                # transpose VT -> V natural; 2 t-tiles of 128 per half.
                # Both transposes into one PSUM tile back-to-back, then the
                # DVE copies drain it (keeps the PE stream dense).
                def th():
                    b = tch // 4
                    vts = _vts[tch]
                    tts = (0, 1) if half == 0 else (2, 3)
                    pvt = psF.tile([P, 256], f32, tag="fx", name="pvt")
                    for j, tt in enumerate(tts):
                        nc.tensor.transpose(
                            pvt[:, ds(j * P, P)], vts[:, ds(tt * P, P)],
                            ident_f,
                        )
                    for j, tt in enumerate(tts):
                        ktile = (tch % 4) * 4 + tt
                        nc.vector.tensor_copy(
                            va_sb[b][:, ds(ktile * 65, DK)],
                            pvt[:, ds(j * P, DK)],
                        )
                        nc.vector.tensor_copy(
                            vb_sb[b][:, ds(ktile * 65, DK)],
                            pvt[:, ds(j * P + DK, DK)],
                        )
                return th

            # ---- the 8 attention chunks as one flat 128-slot pipeline ----
            CHUNKS = [(b, qch) for b in range(NB) for qch in range(4)]
            cstate = [dict() for _ in CHUNKS]

            def fin_thunks(ci):
                # finalize chunk ci: lazy thunks reading cstate[ci], which is
                # populated at the chunk boundary (ua/ub/den0 staged there).
                st = cstate[ci]
                b, qch = CHUNKS[ci]

                def R():
                    # den0[0, 0:512|512:1024] = softmax denominators of both
                    # heads, DMA-staged to partition 0 at the boundary
                    rf = smallp.tile([P, 1024], f32, tag="rf", name="rf")
                    nc.vector.reciprocal_approx_fast(
                        out=rf[0:1, :], in_=st["den0"][0:1, :])
                    rr = smallp.tile([P, 1024], f16, tag="rr", name="rr")
                    nc.vector.tensor_copy(rr[0:1, :], rf[0:1, :])
                    st["rr"] = rr
                    if debug and ci == 0:
                        nc.sync.dma_start(dbg["rr"], rr[0:1, :])

                def bc2():
                    for which in range(2):
                        ps = psF.tile([P, 512], f32, tag="fx", name="bc")
                        nc.tensor.matmul(
                            ps[0:DK, :], ones16[0:1, 0:DK],
                            st["rr"][0:1, ds(which * 512, 512)],
                            start=True, stop=True,
                        )
                        st[f"bc{which}"] = ps

                def mul2():
                    ctq = ctqp.tile([P, 512], f16, tag="ctq", name="ctq")
                    nc.vector.tensor_mul(
                        ctq[0:DK, :], st["ua"][0:DK, :], st["bc0"][0:DK, :])
                    st["ctq"] = ctq
                    tmpb = bsbp.tile([DK, 512], f16, tag="tmpb", name="tmpb")
                    nc.vector.tensor_mul(
                        tmpb, st["ub"][0:DK, :], st["bc1"][0:DK, :])
                    nc.sync.dma_start(st["ctq"][DK:P, :], tmpb)

                def op(tt):
                    def th():
                        ctq = st["ctq"]
                        if debug and ci == 0 and tt == 0:
                            nc.sync.dma_start(dbg["ctq"], ctq)
                        po_sb = posbp.tile([P, 1024], f16, tag="po",
                                           name="po_sb")
                        pos = []
                        for half in range(2):
                            po = psF.tile([P, 512], f32, tag="fx", name="po")
                            nc.tensor.matmul(
                                po, ctq[:, ds(tt * P, P)],
                                wo_sb[:, ds(half * 512, 512)],
                                start=True, stop=True,
                            )
                            pos.append(po)
                        nc.vector.tensor_copy(po_sb[:, 0:512], pos[0])
                        if ci == len(CHUNKS) - 1:
                            nc.scalar.copy(po_sb[:, 512:1024], pos[1])
                        else:
                            nc.vector.tensor_copy(po_sb[:, 512:1024], pos[1])
                        r0 = b * T + qch * 512 + tt * P
                        nc.sync.dma_start(out[r0: r0 + P, :], po_sb)
                    return th

                return [R, bc2, mul2, op(0), op(1), op(2), op(3)]

            def ctx_mm(ci, kt):
                b, qch = CHUNKS[ci]
                st = cstate[ci]
                e = st["escs"].pop(kt)
                nc.tensor.matmul(
                    st["cxa"], va_sb[b][:, ds(kt * 65, 65)], e[:, 0:512],
                    start=(kt == 0), stop=(kt == KT - 1),
                )
                nc.tensor.matmul(
                    st["cxb"], vb_sb[b][:, ds(kt * 65, 65)], e[:, 512:1024],
                    start=(kt == 0), stop=(kt == KT - 1),
                )

            def boundary(ci):
                # close chunk ci: last ctx, evict accumulators, stage the
                # denominator rows (partition 64) to partition 0 via DMA
                st = cstate[ci]
                ctx_mm(ci, KT - 1)
                ua = bsbp.tile([65, 512], f32, tag="ua", name="ua")
                nc.vector.tensor_copy(ua, st["cxa"])
                ub = bsbp.tile([65, 512], f32, tag="ub", name="ub")
                nc.vector.tensor_copy(ub, st["cxb"])
                st["ua"], st["ub"] = ua, ub
                den0 = smallp.tile([1, 1024], f32, tag="den0", name="den0")
                nc.sync.dma_start(den0[0:1, 0:512], ua[64:65, :])
                nc.sync.dma_start(den0[0:1, 512:1024], ub[64:65, :])
                st["den0"] = den0
                if debug and ci == 0:
                    nc.sync.dma_start(dbg["ua"], ua)
                    nc.sync.dma_start(dbg["ub"], ub)

            sc_ready = {}

            def emit_scores(ci, kt):
                # scores run one slot ahead of their exp so the scalar
                # engine's sem wait is always pre-satisfied
                b, qch = CHUNKS[ci]
                q0 = qch * 512
                sc = psS.tile([P, 1024], f32, tag="sc", name="sc")
                nc.tensor.matmul(
                    sc[:, 0:512],
                    kt_sb[b][0:DK, ds(kt * P, P)],
                    qt_sb[b][0:DK, ds(q0, 512)],
                    start=True, stop=True,
                )
                nc.tensor.matmul(
                    sc[:, 512:1024],
                    kt_sb[b][DK:P, ds(kt * P, P)],
                    qt_sb[b][DK:P, ds(q0, 512)],
                    start=True, stop=True,
                    tile_position=(64, 0),
                )
                sc_ready[(ci, kt)] = sc

            def run_chunk(ci, drains):
                b, qch = CHUNKS[ci]
                st = cstate[ci]
                st["cxa"] = psC.tile([65, 512], f32, tag="cx", name="cxa")
                st["cxb"] = psC.tile([65, 512], f32, tag="cx", name="cxb")
                st["escs"] = {}
                for kt in range(KT):
                    sc = sc_ready.pop((ci, kt))
                    esc = escp.tile([P, 1024], f16, tag="esc", name="esc")
                    nc.scalar.activation(esc, sc, Exp, scale=0.125)
                    st["escs"][kt] = esc
                    if kt < KT - 1:
                        emit_scores(ci, kt + 1)
                    elif ci + 1 < len(CHUNKS):
                        emit_scores(ci + 1, 0)
                    if kt == 0:
                        if ci > 0:
                            boundary(ci - 1)
                    else:
                        ctx_mm(ci, kt - 1)
                    for th in drains[kt]:
                        th()

            def sched(*slots):
                d = [[] for _ in range(KT)]
                for i, s in enumerate(slots):
                    if s:
                        d[i] = list(s) if isinstance(s, (list, tuple)) else [s]
                return d

            # brief HAM warmup, then K0 emitted as a single 8-chunk
            # accumulation: each matmul depends only on its own x0 quarter,
            # so the first one starts as soon as 128KB of x has landed
            for w in range(8):
                wt = psS.tile([P, 1024], f32, tag="sc", name="warm")
                nc.tensor.matmul(wt[:, 0:512], ident, ones16,
                                 start=True, stop=True)
            ps0 = psF.tile([P, 512], f32, tag="fx", name="pj0")
            for c in range(DC):
                nc.tensor.matmul(ps0, wk_sb[:, c], x0[:, c],
                                 start=(c == 0), stop=(c == DC - 1))
            nc.vector.tensor_scalar_add(kt_sb[0][:, 0:512], ps0, bk_sb)
            Q(0, 0)(); Q(0, 1)()
            V(0, 0)(); V(0, 1)()
            Vt(0, 0)(); Vt(0, 1)()
            emit_scores(0, 0)

            # chunk 0: remaining b0 projections (x1 load issued in head
            # epilogue below; x2/x3/wo issues spread through the chunk)
            run_chunk(0, sched(
                [K(1, 0), K(1, 1)], [V(1, 0), V(1, 1)], [Vt(1, 0), Vt(1, 1)],
                [load_x(3), K(2, 0)], K(2, 1), [V(2, 0), V(2, 1)],
                [Vt(2, 0), Vt(2, 1)],
                K(3, 0), K(3, 1), V(3, 0), V(3, 1), [load_wo, Vt(3, 0)],
                Vt(3, 1), Q(1, 0), Q(1, 1), None,
            ))
            f = fin_thunks(0)
            run_chunk(1, sched(
                [load_x(4), f[0]], f[1], f[2], Q(2, 0), Q(2, 1), f[3],
                K(4, 0), K(4, 1), f[4], [load_x(5), V(4, 0)], V(4, 1),
                f[5], f[6], [Vt(4, 0), Vt(4, 1)], None, None,
            ))
            f = fin_thunks(1)
            run_chunk(2, sched(
                [Q(3, 0), f[0]], Q(3, 1), f[1], f[2], Q(4, 0), Q(4, 1),
                f[3], [load_x(6), K(5, 0)], K(5, 1), f[4], V(5, 0), V(5, 1),
                f[5], f[6], [Vt(5, 0), Vt(5, 1)], None,
            ))
            f = fin_thunks(2)
            run_chunk(3, sched(
                f[0], f[1], f[2], [load_x(7), K(6, 0)], K(6, 1), f[3],
                V(6, 0), V(6, 1), f[4], [Vt(6, 0), Vt(6, 1)],
                K(7, 0), K(7, 1), f[5], V(7, 0), V(7, 1), f[6],
            ))
            f = fin_thunks(3)
            run_chunk(4, sched(
                [Vt(7, 0), Vt(7, 1), f[0]], f[1], f[2], Q(5, 0), Q(5, 1),
                f[3], f[4], f[5], f[6], None, None, None, None, None,
                None, None,
            ))
            f = fin_thunks(4)
            run_chunk(5, sched(
                f[0], f[1], f[2], Q(6, 0), Q(6, 1), f[3], f[4], f[5], f[6],
                None, None, None, None, None, None, None,
            ))
            f = fin_thunks(5)
            run_chunk(6, sched(
                f[0], f[1], f[2], Q(7, 0), Q(7, 1), f[3], f[4], f[5], f[6],
                None, None, None, None, None, None, None,
            ))
            f = fin_thunks(6)
            run_chunk(7, sched(
                f[0], f[1], f[2], f[3], f[4], f[5], f[6],
                None, None, None, None, None, None, None, None, None,
            ))
            # tail: close and finalize the last chunk
            boundary(7)
            for th in fin_thunks(7):
                th()
            if debug:
                nc.sync.dma_start(dbg["qt"], qt_sb[0])
                nc.sync.dma_start(dbg["kt"], kt_sb[0])

    nc.compile()
    return nc


def _get_nc(reps=1, debug=False):
    key = f"nc{reps}_{debug}"
    if key not in _CACHE:
        _CACHE[key] = _build(reps, debug=debug)
    return _CACHE[key]


def kernel(x, Wq, bq, Wk, bk, Wv, bv, Wo, bo):
    from concourse.bass_utils import run_bass_kernel_spmd

    x = np.asarray(x, dtype=np.float32)
    Wq = np.asarray(Wq, dtype=np.float32)
    Wk = np.asarray(Wk, dtype=np.float32)
    Wv = np.asarray(Wv, dtype=np.float32)
    Wo = np.asarray(Wo, dtype=np.float32)
    bq = np.asarray(bq, dtype=np.float32)
    bk = np.asarray(bk, dtype=np.float32)
    bv = np.asarray(bv, dtype=np.float32)
    bo = np.asarray(bo, dtype=np.float32)

    B, Tl, Dl = x.shape
    xt = np.ascontiguousarray(x.reshape(B * Tl, Dl).T.astype(np.float16))

    in_maps = []
    for c in range(NCORES):
        sl = slice(c * P, (c + 1) * P)
        in_maps.append(
            {
                "xt": xt,
                "wq": np.ascontiguousarray(Wq[sl, :].T.astype(np.float16)),
                "wk": np.ascontiguousarray(Wk[sl, :].T.astype(np.float16)),
                "wv": np.ascontiguousarray(Wv[sl, :].T.astype(np.float16)),
                "wo": np.ascontiguousarray(Wo[:, sl].T.astype(np.float16)),
                "bq": np.ascontiguousarray(bq[sl].reshape(P, 1)),
                "bk": np.ascontiguousarray(bk[sl].reshape(P, 1)),
                "bv": np.ascontiguousarray(bv[sl].reshape(P, 1)),
            }
        )

    nc = _get_nc()
    _CACHE["in_maps"] = in_maps
    res = run_bass_kernel_spmd(nc, in_maps, core_ids=list(range(NCORES)))
    acc = res.results[0]["out"].astype(np.float32)
    for c in range(1, NCORES):
        acc = acc + res.results[c]["out"].astype(np.float32)
    acc = acc + bo[None, :]
    return acc.reshape(B, Tl, Dl).astype(np.float32)


# revision 27
# speedup vs baseline: 1.1955x; 1.1955x over previous
"""Trainium2 Bass kernel for nn_MultiHeadAttention (B=2, T=2048, D=1024, H=16).

Sharding: 8 cores; core c owns head pair (2c, 2c+1) = output-channel slice
[c*128, (c+1)*128) of Wq/Wk/Wv columns and Wo rows (tensor parallel), both
batches. Host pre-transposes x and weight slices (cast to f16); each core
computes a partial output projection over its 128 ctx channels in f16; host
sums the 8 partials in f32 (replaces the all-reduce) and adds bo.

The kernel is one dense exp stream on the scalar engine (the binding engine
at ~1.0us per [128,1024] score tile, 128 tiles) with all other work
(projections, V transposes, out-projection, softmax normalization) scheduled
into per-ktile drain slots under it. The 8 attention chunks form a single
flat 128-slot pipeline: scores(kt) -> exp(kt) -> ctx(kt) with scores one
slot ahead of ctx, continuing seamlessly across chunk boundaries; the
previous chunk's finalize (reciprocal of the softmax denominators staged to
partition 0 via a small DMA, PE broadcast, normalize, out-projection) is
drained through the following chunk's slots.
"""

import numpy as np

P = 128
D = 1024
BT = 4096
T = 2048
NB = 2
DC = 8    # D chunks of 128
KT = 16   # 128-wide k-tiles per batch
NCORES = 8
DK = 64

_CACHE = {}


def _build(reps=1, debug=False):
    import concourse.bass as bass
    import concourse.tile as tile
    from concourse import bacc, mybir
    from concourse.masks import make_identity

    f32 = mybir.dt.float32
    f16 = mybir.dt.float16
    f8 = mybir.dt.float8e4
    DR = mybir.MatmulPerfMode.DoubleRow
    Exp = mybir.ActivationFunctionType.Exp
    ds = bass.ds

    nc = bacc.Bacc("TRN2", target_bir_lowering=False, debug=False)

    xt = nc.dram_tensor("xt", [D, BT], f16, kind="ExternalInput").ap()
    wq = nc.dram_tensor("wq", [D, P], f16, kind="ExternalInput").ap()
    wk = nc.dram_tensor("wk", [D, P], f16, kind="ExternalInput").ap()
    wv = nc.dram_tensor("wv", [D, P], f16, kind="ExternalInput").ap()
    wo = nc.dram_tensor("wo", [P, D], f16, kind="ExternalInput").ap()
    bqd = nc.dram_tensor("bq", [P, 1], f32, kind="ExternalInput").ap()
    bkd = nc.dram_tensor("bk", [P, 1], f32, kind="ExternalInput").ap()
    bvd = nc.dram_tensor("bv", [P, 1], f32, kind="ExternalInput").ap()
    out = nc.dram_tensor("out", [BT, D], f16, kind="ExternalOutput").ap()
    dbg = {}
    if debug:
        dbg["qt"] = nc.dram_tensor("dbg_qt", [P, T], f16, kind="ExternalOutput").ap()
        dbg["kt"] = nc.dram_tensor("dbg_kt", [P, T], f16, kind="ExternalOutput").ap()
        dbg["va"] = nc.dram_tensor("dbg_va", [P, KT * 65], f16, kind="ExternalOutput").ap()
        dbg["vb"] = nc.dram_tensor("dbg_vb", [P, KT * 65], f16, kind="ExternalOutput").ap()
        dbg["esc"] = nc.dram_tensor("dbg_esc", [P, 1024], f16, kind="ExternalOutput").ap()
        dbg["ua"] = nc.dram_tensor("dbg_ua", [65, 512], f32, kind="ExternalOutput").ap()
        dbg["ub"] = nc.dram_tensor("dbg_ub", [65, 512], f32, kind="ExternalOutput").ap()
        dbg["rr"] = nc.dram_tensor("dbg_rr", [1, 1024], f16, kind="ExternalOutput").ap()
        dbg["ctq"] = nc.dram_tensor("dbg_ctq", [P, 512], f16, kind="ExternalOutput").ap()

    with tile.TileContext(nc) as tc:
        with (
            tc.tile_pool(name="const", bufs=1) as constp,
            tc.tile_pool(name="xtp", bufs=3) as xtp,
            tc.tile_pool(name="qkv", bufs=1) as qkvp,
            tc.tile_pool(name="vts", bufs=2) as vtsp,
            tc.tile_pool(name="esc", bufs=3) as escp,
            tc.tile_pool(name="ctq", bufs=2) as ctqp,
            tc.tile_pool(name="small", bufs=2) as smallp,
            tc.tile_pool(name="bsb", bufs=2) as bsbp,
            tc.tile_pool(name="posb", bufs=3) as posbp,
            # PSUM: sc 2x2 banks + cx 2x1 banks + flex 2x1 banks = 8
            tc.tile_pool(name="psS", bufs=2, space="PSUM") as psS,
            tc.tile_pool(name="psC", bufs=2, space="PSUM") as psC,
            tc.tile_pool(name="psF", bufs=2, space="PSUM") as psF,
        ):
            # ---- constants / weights; DMA order puts wk and x(0) first so
            # the first projection can start as early as possible ----
            wq_sb = constp.tile([P, DC, P], f16, tag="wq")
            wk_sb = constp.tile([P, DC, P], f16, tag="wk")
            wv_sb = constp.tile([P, DC, P], f16, tag="wv")
            wo_sb = constp.tile([P, D], f16, tag="wo")
            bq_sb = constp.tile([P, 1], f32, tag="bq")
            bk_sb = constp.tile([P, 1], f32, tag="bk")
            bv_sb = constp.tile([P, 1], f32, tag="bv")
            ident_f = constp.tile([P, P], f32, tag="identf")
            make_identity(nc, ident_f)
            ident = constp.tile([P, P], f16, tag="ident")
            nc.vector.tensor_copy(ident, ident_f)
            ones16 = constp.tile([P, 512], f16, tag="ones16")
            nc.vector.memset(ones16, 1.0)

            xt_r = xt.rearrange("(c p) t -> p c t", p=P)
            xtiles = {}

            def load_x(tch, eng=None):
                def th():
                    t0 = xtp.tile([P, DC, 512], f16, tag="xt", name=f"x{tch}")
                    (eng or nc.gpsimd).dma_start(
                        t0, xt_r[:, :, ds(tch * 512, 512)])
                    xtiles[tch] = t0
                return th

            # Head DMA: transfers on one ring are FIFO, so the sync ring
            # carries x0 (split) then x1/x2 in priority order while the
            # gpsimd ring carries the weights; biases ride the scalar queue.
            # x3..x7 are gated behind compute by the 3-deep x pool rotation.
            x0 = xtp.tile([P, DC, 512], f16, tag="xt", name="x0")
            for qq in range(4):
                nc.sync.dma_start(x0[:, 2 * qq: 2 * qq + 2],
                                  xt_r[:, 2 * qq: 2 * qq + 2, ds(0, 512)])
            xtiles[0] = x0
            nc.gpsimd.dma_start(wk_sb, wk.rearrange("(c p) e -> p c e", p=P))
            nc.scalar.dma_start(bk_sb, bkd)
            nc.scalar.dma_start(bq_sb, bqd)
            nc.scalar.dma_start(bv_sb, bvd)
            nc.gpsimd.dma_start(wq_sb, wq.rearrange("(c p) e -> p c e", p=P))
            load_x(1, nc.sync)()
            nc.gpsimd.dma_start(wv_sb, wv.rearrange("(c p) e -> p c e", p=P))
            load_x(2, nc.sync)()

            def load_wo():
                nc.sync.dma_start(wo_sb, wo)

            # ---- per-batch persistent tiles ----
            qt_sb = [
                qkvp.tile([P, T], f16, tag=f"qt{b}", name=f"qt{b}")
                for b in range(NB)
            ]
            kt_sb = [
                qkvp.tile([P, T], f16, tag=f"kt{b}", name=f"kt{b}")
                for b in range(NB)
            ]
            # V natural per batch, 65-wide blocks per ktile: [V(64)|1]; the
            # ones column accumulates the softmax denominator for free.
            # (fp8 DoubleRow ctx was tried: ~27us faster on PE but the fp8
            # V-quantization noise on peaked attention rows sums to ~2.5-4%
            # across the 8 cores' partials -- over the tolerance.)
            va_sb = [
                qkvp.tile([P, KT * 65], f16, tag=f"va{b}", name=f"va{b}")
                for b in range(NB)
            ]
            vb_sb = [
                qkvp.tile([P, KT * 65], f16, tag=f"vb{b}", name=f"vb{b}")
                for b in range(NB)
            ]
            ones_col = ones16[:, 0:KT].rearrange("p (k one) -> p k one", one=1)
            for b in range(NB):
                nc.vector.tensor_copy(
                    va_sb[b].rearrange("p (k c) -> p k c", c=65)[:, :, 64:65],
                    ones_col,
                )
                nc.vector.tensor_copy(
                    vb_sb[b].rearrange("p (k c) -> p k c", c=65)[:, :, 64:65],
                    ones_col,
                )

            _proj_ps = {}

            def proj_half(tch, w_sb, b_sb, dst, half):
                # half 0: open PSUM accumulation, 4 contraction chunks;
                # half 1: 4 more chunks, close group, evict (bias+cast f16)
                def th():
                    if half == 0:
                        ps = psF.tile([P, 512], f32, tag="fx", name="pj")
                        _proj_ps[(tch, id(w_sb))] = ps
                        for c in range(4):
                            nc.tensor.matmul(
                                ps, w_sb[:, c], xtiles[tch][:, c],
                                start=(c == 0), stop=False,
                            )
                    else:
                        ps = _proj_ps.pop((tch, id(w_sb)))
                        for c in range(4, DC):
                            nc.tensor.matmul(
                                ps, w_sb[:, c], xtiles[tch][:, c],
                                start=False, stop=(c == DC - 1),
                            )
                        nc.vector.tensor_scalar_add(dst, ps, b_sb)
                return th

            def K(tch, half):
                b = tch // 4
                dst = kt_sb[b][:, ds((tch % 4) * 512, 512)]
                return proj_half(tch, wk_sb, bk_sb, dst, half)

            def Q(tch, half):
                b = tch // 4
                dst = qt_sb[b][:, ds((tch % 4) * 512, 512)]
                return proj_half(tch, wq_sb, bq_sb, dst, half)

            _vts = {}

            def V(tch, half):
                def th():
                    if half == 0:
                        _vts[tch] = vtsp.tile([P, 512], f32, tag="vts",
                                              name=f"v{tch}")
                    proj_half(tch, wv_sb, bv_sb, _vts[tch], half)()
                return th

            def Vt(tch, half):
                # transpose VT -> V natural; 2 t-tiles of 128 per half.
                # Both transposes into one PSUM tile back-to-back, then the
                # DVE copies drain it (keeps the PE stream dense).
                def th():
                    b = tch // 4
                    vts = _vts[tch]
                    tts = (0, 1) if half == 0 else (2, 3)
                    pvt = psF.tile([P, 256], f32, tag="fx", name="pvt")
                    for j, tt in enumerate(tts):
                        nc.tensor.transpose(
                            pvt[:, ds(j * P, P)], vts[:, ds(tt * P, P)],
                            ident_f,
                        )
                    for j, tt in enumerate(tts):
                        ktile = (tch % 4) * 4 + tt
                        nc.vector.tensor_copy(
                            va_sb[b][:, ds(ktile * 65, DK)],
                            pvt[:, ds(j * P, DK)],
                        )
                        nc.vector.tensor_copy(
                            vb_sb[b][:, ds(ktile * 65, DK)],
                            pvt[:, ds(j * P + DK, DK)],
                        )
                return th

            # ---- the 8 attention chunks as one flat 128-slot pipeline ----
            CHUNKS = [(b, qch) for b in range(NB) for qch in range(4)]
            cstate = [dict() for _ in CHUNKS]

            def fin_thunks(ci):
                # finalize chunk ci: lazy thunks reading cstate[ci], which is
                # populated at the chunk boundary (ua/ub/den0 staged there).
                st = cstate[ci]
                b, qch = CHUNKS[ci]

                def R():
                    # den0[0, 0:512|512:1024] = softmax denominators of both
                    # heads, DMA-staged to partition 0 at the boundary
                    rf = smallp.tile([P, 1024], f32, tag="rf", name="rf")
                    nc.vector.reciprocal_approx_fast(
                        out=rf[0:1, :], in_=st["den0"][0:1, :])
                    rr = smallp.tile([P, 1024], f16, tag="rr", name="rr")
                    nc.vector.tensor_copy(rr[0:1, :], rf[0:1, :])
                    st["rr"] = rr
                    if debug and ci == 0:
                        nc.sync.dma_start(dbg["rr"], rr[0:1, :])

                def bc2():
                    for which in range(2):
                        ps = psF.tile([P, 512], f32, tag="fx", name="bc")
                        nc.tensor.matmul(
                            ps[0:DK, :], ones16[0:1, 0:DK],
                            st["rr"][0:1, ds(which * 512, 512)],
                            start=True, stop=True,
                        )
                        st[f"bc{which}"] = ps

                def mul2():
                    ctq = ctqp.tile([P, 512], f16, tag="ctq", name="ctq")
                    nc.vector.tensor_mul(
                        ctq[0:DK, :], st["ua"][0:DK, :], st["bc0"][0:DK, :])
                    st["ctq"] = ctq
                    tmpb = bsbp.tile([DK, 512], f16, tag="tmpb", name="tmpb")
                    nc.vector.tensor_mul(
                        tmpb, st["ub"][0:DK, :], st["bc1"][0:DK, :])
                    nc.sync.dma_start(st["ctq"][DK:P, :], tmpb)

                def op(tt):
                    def th():
                        ctq = st["ctq"]
                        if debug and ci == 0 and tt == 0:
                            nc.sync.dma_start(dbg["ctq"], ctq)
                        po_sb = posbp.tile([P, 1024], f16, tag="po",
                                           name="po_sb")
                        pos = []
                        for half in range(2):
                            po = psF.tile([P, 512], f32, tag="fx", name="po")
                            nc.tensor.matmul(
                                po, ctq[:, ds(tt * P, P)],
                                wo_sb[:, ds(half * 512, 512)],
                                start=True, stop=True,
                            )
                            pos.append(po)
                        nc.vector.tensor_copy(po_sb[:, 0:512], pos[0])
                        if ci == len(CHUNKS) - 1:
                            nc.scalar.copy(po_sb[:, 512:1024], pos[1])
                        else:
                            nc.vector.tensor_copy(po_sb[:, 512:1024], pos[1])
                        r0 = b * T + qch * 512 + tt * P
                        # out partials ride the gpsimd ring so the sync ring
                        # stays clear for the latency-critical den0/tmpb
                        # staging transfers (op matmuls wait on tmpb's ctq
                        # write); tail alternates rings to drain faster
                        if ci == len(CHUNKS) - 1 and tt % 2 == 0:
                            nc.sync.dma_start(out[r0: r0 + P, :], po_sb)
                        else:
                            nc.gpsimd.dma_start(out[r0: r0 + P, :], po_sb)
                    return th

                return [R, bc2, mul2, op(0), op(1), op(2), op(3)]

            def ctx_mm(ci, kt):
                b, qch = CHUNKS[ci]
                st = cstate[ci]
                e = st["escs"].pop(kt)
                nc.tensor.matmul(
                    st["cxa"], va_sb[b][:, ds(kt * 65, 65)], e[:, 0:512],
                    start=(kt == 0), stop=(kt == KT - 1),
                )
                nc.tensor.matmul(
                    st["cxb"], vb_sb[b][:, ds(kt * 65, 65)], e[:, 512:1024],
                    start=(kt == 0), stop=(kt == KT - 1),
                )

            def boundary(ci):
                # close chunk ci: last ctx, evict accumulators, stage the
                # denominator rows (partition 64) to partition 0 via DMA
                st = cstate[ci]
                ctx_mm(ci, KT - 1)
                ua = bsbp.tile([65, 512], f32, tag="ua", name="ua")
                nc.vector.tensor_copy(ua, st["cxa"])
                ub = bsbp.tile([65, 512], f32, tag="ub", name="ub")
                nc.vector.tensor_copy(ub, st["cxb"])
                st["ua"], st["ub"] = ua, ub
                den0 = smallp.tile([1, 1024], f32, tag="den0", name="den0")
                nc.sync.dma_start(den0[0:1, 0:512], ua[64:65, :])
                nc.sync.dma_start(den0[0:1, 512:1024], ub[64:65, :])
                st["den0"] = den0
                if debug and ci == 0:
                    nc.sync.dma_start(dbg["ua"], ua)
                    nc.sync.dma_start(dbg["ub"], ub)

            sc_ready = {}

            def emit_scores(ci, kt):
                # scores run one slot ahead of their exp so the scalar
                # engine's sem wait is always pre-satisfied
                b, qch = CHUNKS[ci]
                q0 = qch * 512
                sc = psS.tile([P, 1024], f32, tag="sc", name="sc")
                nc.tensor.matmul(
                    sc[:, 0:512],
                    kt_sb[b][0:DK, ds(kt * P, P)],
                    qt_sb[b][0:DK, ds(q0, 512)],
                    start=True, stop=True,
                )
                nc.tensor.matmul(
                    sc[:, 512:1024],
                    kt_sb[b][DK:P, ds(kt * P, P)],
                    qt_sb[b][DK:P, ds(q0, 512)],
                    start=True, stop=True,
                    tile_position=(64, 0),
                )
                sc_ready[(ci, kt)] = sc

            def run_chunk(ci, drains):
                b, qch = CHUNKS[ci]
                st = cstate[ci]
                st["cxa"] = psC.tile([65, 512], f32, tag="cx", name="cxa")
                st["cxb"] = psC.tile([65, 512], f32, tag="cx", name="cxb")
                st["escs"] = {}
                for kt in range(KT):
                    sc = sc_ready.pop((ci, kt))
                    esc = escp.tile([P, 1024], f16, tag="esc", name="esc")
                    nc.scalar.activation(esc, sc, Exp, scale=0.125)
                    st["escs"][kt] = esc
                    if kt < KT - 1:
                        emit_scores(ci, kt + 1)
                    elif ci + 1 < len(CHUNKS):
                        emit_scores(ci + 1, 0)
                    if kt == 0:
                        if ci > 0:
                            boundary(ci - 1)
                    else:
                        ctx_mm(ci, kt - 1)
                    for th in drains[kt]:
                        th()

            def sched(*slots):
                d = [[] for _ in range(KT)]
                for i, s in enumerate(slots):
                    if s:
                        d[i] = list(s) if isinstance(s, (list, tuple)) else [s]
                return d

            # brief HAM warmup, then K0 emitted as a single 8-chunk
            # accumulation: each matmul depends only on its own x0 quarter,
            # so the first one starts as soon as 128KB of x has landed
            for w in range(3):
                wt = psS.tile([P, 1024], f32, tag="sc", name="warm")
                nc.tensor.matmul(wt[:, 0:512], ident, ones16,
                                 start=True, stop=True)
            ps0 = psF.tile([P, 512], f32, tag="fx", name="pj0")
            for c in range(DC):
                nc.tensor.matmul(ps0, wk_sb[:, c], x0[:, c],
                                 start=(c == 0), stop=(c == DC - 1))
            nc.vector.tensor_scalar_add(kt_sb[0][:, 0:512], ps0, bk_sb)
            Q(0, 0)(); Q(0, 1)()
            V(0, 0)(); V(0, 1)()
            Vt(0, 0)(); Vt(0, 1)()
            emit_scores(0, 0)

            # chunk 0: remaining b0 projections (x1 load issued in head
            # epilogue below; x2/x3/wo issues spread through the chunk)
            run_chunk(0, sched(
                [K(1, 0), K(1, 1)], [V(1, 0), V(1, 1)], [Vt(1, 0), Vt(1, 1)],
                [load_x(3), K(2, 0), K(2, 1)], [V(2, 0), V(2, 1)],
                [Vt(2, 0), Vt(2, 1)],
                K(3, 0), K(3, 1), V(3, 0), V(3, 1), [load_wo, Vt(3, 0)],
                Vt(3, 1), Q(1, 0), Q(1, 1), None, None,
            ))
            f = fin_thunks(0)
            run_chunk(1, sched(
                [load_x(4), f[0]], f[1], f[2], Q(2, 0), Q(2, 1), f[3],
                K(4, 0), K(4, 1), f[4], [load_x(5), V(4, 0)], V(4, 1),
                f[5], f[6], [Vt(4, 0), Vt(4, 1)], None, None,
            ))
            f = fin_thunks(1)
            run_chunk(2, sched(
                [Q(3, 0), f[0]], Q(3, 1), f[1], f[2], Q(4, 0), Q(4, 1),
                f[3], [load_x(6), K(5, 0)], K(5, 1), f[4], V(5, 0), V(5, 1),
                f[5], f[6], [Vt(5, 0), Vt(5, 1)], None,
            ))
            f = fin_thunks(2)
            run_chunk(3, sched(
                f[0], f[1], f[2], [load_x(7), K(6, 0)], K(6, 1), f[3],
                V(6, 0), V(6, 1), f[4], [Vt(6, 0), Vt(6, 1)],
                K(7, 0), K(7, 1), f[5], V(7, 0), V(7, 1), f[6],
            ))
            f = fin_thunks(3)
            run_chunk(4, sched(
                [Vt(7, 0), Vt(7, 1), f[0]], f[1], f[2], Q(5, 0), Q(5, 1),
                f[3], f[4], f[5], f[6], None, None, None, None, None,
                None, None,
            ))
            f = fin_thunks(4)
            run_chunk(5, sched(
                f[0], f[1], f[2], Q(6, 0), Q(6, 1), f[3], f[4], f[5], f[6],
                None, None, None, None, None, None, None,
            ))
            f = fin_thunks(5)
            run_chunk(6, sched(
                f[0], f[1], f[2], Q(7, 0), Q(7, 1), f[3], f[4], f[5], f[6],
                None, None, None, None, None, None, None,
            ))
            f = fin_thunks(6)
            run_chunk(7, sched(
                f[0], f[1], f[2], f[3], f[4], f[5], f[6],
                None, None, None, None, None, None, None, None, None,
            ))
            # tail: close and finalize the last chunk
            boundary(7)
            for th in fin_thunks(7):
                th()
            if debug:
                nc.sync.dma_start(dbg["qt"], qt_sb[0])
                nc.sync.dma_start(dbg["kt"], kt_sb[0])

    nc.compile()
    return nc


def _get_nc(reps=1, debug=False):
    key = f"nc{reps}_{debug}"
    if key not in _CACHE:
        _CACHE[key] = _build(reps, debug=debug)
    return _CACHE[key]


def kernel(x, Wq, bq, Wk, bk, Wv, bv, Wo, bo):
    from concourse.bass_utils import run_bass_kernel_spmd

    x = np.asarray(x, dtype=np.float32)
    Wq = np.asarray(Wq, dtype=np.float32)
    Wk = np.asarray(Wk, dtype=np.float32)
    Wv = np.asarray(Wv, dtype=np.float32)
    Wo = np.asarray(Wo, dtype=np.float32)
    bq = np.asarray(bq, dtype=np.float32)
    bk = np.asarray(bk, dtype=np.float32)
    bv = np.asarray(bv, dtype=np.float32)
    bo = np.asarray(bo, dtype=np.float32)

    B, Tl, Dl = x.shape
    xt = np.ascontiguousarray(x.reshape(B * Tl, Dl).T.astype(np.float16))

    in_maps = []
    for c in range(NCORES):
        sl = slice(c * P, (c + 1) * P)
        in_maps.append(
            {
                "xt": xt,
                "wq": np.ascontiguousarray(Wq[sl, :].T.astype(np.float16)),
                "wk": np.ascontiguousarray(Wk[sl, :].T.astype(np.float16)),
                "wv": np.ascontiguousarray(Wv[sl, :].T.astype(np.float16)),
                "wo": np.ascontiguousarray(Wo[:, sl].T.astype(np.float16)),
                "bq": np.ascontiguousarray(bq[sl].reshape(P, 1)),
                "bk": np.ascontiguousarray(bk[sl].reshape(P, 1)),
                "bv": np.ascontiguousarray(bv[sl].reshape(P, 1)),
            }
        )

    nc = _get_nc()
    _CACHE["in_maps"] = in_maps
    res = run_bass_kernel_spmd(nc, in_maps, core_ids=list(range(NCORES)))
    acc = res.results[0]["out"].astype(np.float32)
    for c in range(1, NCORES):
        acc = acc + res.results[c]["out"].astype(np.float32)
    acc = acc + bo[None, :]
    return acc.reshape(B, Tl, Dl).astype(np.float32)


# revision 29
# speedup vs baseline: 1.1989x; 1.0029x over previous
"""Trainium2 Bass kernel for nn_MultiHeadAttention (B=2, T=2048, D=1024, H=16).

Sharding: 8 cores; core c owns head pair (2c, 2c+1) = output-channel slice
[c*128, (c+1)*128) of Wq/Wk/Wv columns and Wo rows (tensor parallel), both
batches. Host pre-transposes x and weight slices (cast to f16); each core
computes a partial output projection over its 128 ctx channels in f16; host
sums the 8 partials in f32 (replaces the all-reduce) and adds bo.

The kernel is one dense exp stream on the scalar engine (the binding engine
at ~1.0us per [128,1024] score tile, 128 tiles) with all other work
(projections, V transposes, out-projection, softmax normalization) scheduled
into per-ktile drain slots under it. The 8 attention chunks form a single
flat 128-slot pipeline: scores(kt) -> exp(kt) -> ctx(kt) with scores one
slot ahead of ctx, continuing seamlessly across chunk boundaries; the
previous chunk's finalize (reciprocal of the softmax denominators staged to
partition 0 via a small DMA, PE broadcast, normalize, out-projection) is
drained through the following chunk's slots.
"""

import numpy as np

P = 128
D = 1024
BT = 4096
T = 2048
NB = 2
DC = 8    # D chunks of 128
KT = 16   # 128-wide k-tiles per batch
NCORES = 8
DK = 64

_CACHE = {}


def _build(reps=1, debug=False):
    import concourse.bass as bass
    import concourse.tile as tile
    from concourse import bacc, mybir
    from concourse.masks import make_identity

    f32 = mybir.dt.float32
    f16 = mybir.dt.float16
    f8 = mybir.dt.float8e4
    DR = mybir.MatmulPerfMode.DoubleRow
    Exp = mybir.ActivationFunctionType.Exp
    ds = bass.ds

    nc = bacc.Bacc("TRN2", target_bir_lowering=False, debug=False)

    xt = nc.dram_tensor("xt", [D, BT], f16, kind="ExternalInput").ap()
    wq = nc.dram_tensor("wq", [D, P], f16, kind="ExternalInput").ap()
    wk = nc.dram_tensor("wk", [D, P], f16, kind="ExternalInput").ap()
    wv = nc.dram_tensor("wv", [D, P], f16, kind="ExternalInput").ap()
    wo = nc.dram_tensor("wo", [P, D], f16, kind="ExternalInput").ap()
    bqd = nc.dram_tensor("bq", [P, 1], f32, kind="ExternalInput").ap()
    bkd = nc.dram_tensor("bk", [P, 1], f32, kind="ExternalInput").ap()
    bvd = nc.dram_tensor("bv", [P, 1], f32, kind="ExternalInput").ap()
    out = nc.dram_tensor("out", [BT, D], f16, kind="ExternalOutput").ap()
    dbg = {}
    if debug:
        dbg["qt"] = nc.dram_tensor("dbg_qt", [P, T], f16, kind="ExternalOutput").ap()
        dbg["kt"] = nc.dram_tensor("dbg_kt", [P, T], f16, kind="ExternalOutput").ap()
        dbg["va"] = nc.dram_tensor("dbg_va", [P, KT * 65], f16, kind="ExternalOutput").ap()
        dbg["vb"] = nc.dram_tensor("dbg_vb", [P, KT * 65], f16, kind="ExternalOutput").ap()
        dbg["esc"] = nc.dram_tensor("dbg_esc", [P, 1024], f16, kind="ExternalOutput").ap()
        dbg["ua"] = nc.dram_tensor("dbg_ua", [65, 512], f32, kind="ExternalOutput").ap()
        dbg["ub"] = nc.dram_tensor("dbg_ub", [65, 512], f32, kind="ExternalOutput").ap()
        dbg["rr"] = nc.dram_tensor("dbg_rr", [1, 1024], f16, kind="ExternalOutput").ap()
        dbg["ctq"] = nc.dram_tensor("dbg_ctq", [P, 512], f16, kind="ExternalOutput").ap()

    with tile.TileContext(nc) as tc:
        with (
            tc.tile_pool(name="const", bufs=1) as constp,
            tc.tile_pool(name="xtp", bufs=3) as xtp,
            tc.tile_pool(name="qkv", bufs=1) as qkvp,
            tc.tile_pool(name="vts", bufs=2) as vtsp,
            tc.tile_pool(name="esc", bufs=3) as escp,
            tc.tile_pool(name="ctq", bufs=2) as ctqp,
            tc.tile_pool(name="small", bufs=2) as smallp,
            tc.tile_pool(name="bsb", bufs=2) as bsbp,
            tc.tile_pool(name="posb", bufs=3) as posbp,
            # PSUM: sc 2x2 banks + cx 2x1 banks + flex 2x1 banks = 8
            tc.tile_pool(name="psS", bufs=2, space="PSUM") as psS,
            tc.tile_pool(name="psC", bufs=2, space="PSUM") as psC,
            tc.tile_pool(name="psF", bufs=2, space="PSUM") as psF,
        ):
            # ---- constants / weights; DMA order puts wk and x(0) first so
            # the first projection can start as early as possible ----
            wq_sb = constp.tile([P, DC, P], f16, tag="wq")
            wk_sb = constp.tile([P, DC, P], f16, tag="wk")
            wv_sb = constp.tile([P, DC, P], f16, tag="wv")
            wo_sb = constp.tile([P, D], f16, tag="wo")
            bq_sb = constp.tile([P, 1], f32, tag="bq")
            bk_sb = constp.tile([P, 1], f32, tag="bk")
            bv_sb = constp.tile([P, 1], f32, tag="bv")
            ident_f = constp.tile([P, P], f32, tag="identf")
            make_identity(nc, ident_f)
            ident = constp.tile([P, P], f16, tag="ident")
            nc.vector.tensor_copy(ident, ident_f)
            ones16 = constp.tile([P, 512], f16, tag="ones16")
            nc.vector.memset(ones16, 1.0)

            xt_r = xt.rearrange("(c p) t -> p c t", p=P)
            xtiles = {}

            def load_x(tch, eng=None):
                def th():
                    t0 = xtp.tile([P, DC, 512], f16, tag="xt", name=f"x{tch}")
                    (eng or nc.gpsimd).dma_start(
                        t0, xt_r[:, :, ds(tch * 512, 512)])
                    xtiles[tch] = t0
                return th

            # Head DMA: transfers on one ring are FIFO, so the sync ring
            # carries x0 (split) then x1/x2 in priority order while the
            # gpsimd ring carries the weights; biases ride the scalar queue.
            # x3..x7 are gated behind compute by the 3-deep x pool rotation.
            x0 = xtp.tile([P, DC, 512], f16, tag="xt", name="x0")
            for qq in range(4):
                nc.sync.dma_start(x0[:, 2 * qq: 2 * qq + 2],
                                  xt_r[:, 2 * qq: 2 * qq + 2, ds(0, 512)])
            xtiles[0] = x0
            nc.gpsimd.dma_start(wk_sb, wk.rearrange("(c p) e -> p c e", p=P))
            nc.scalar.dma_start(bk_sb, bkd)
            nc.scalar.dma_start(bq_sb, bqd)
            nc.scalar.dma_start(bv_sb, bvd)
            nc.gpsimd.dma_start(wq_sb, wq.rearrange("(c p) e -> p c e", p=P))
            load_x(1, nc.sync)()
            nc.gpsimd.dma_start(wv_sb, wv.rearrange("(c p) e -> p c e", p=P))
            load_x(2, nc.sync)()

            def load_wo():
                nc.sync.dma_start(wo_sb, wo)

            # ---- per-batch persistent tiles ----
            qt_sb = [
                qkvp.tile([P, T], f16, tag=f"qt{b}", name=f"qt{b}")
                for b in range(NB)
            ]
            kt_sb = [
                qkvp.tile([P, T], f16, tag=f"kt{b}", name=f"kt{b}")
                for b in range(NB)
            ]
            # V natural per batch, 65-wide blocks per ktile: [V(64)|1]; the
            # ones column accumulates the softmax denominator for free.
            # (fp8 DoubleRow ctx was tried: ~27us faster on PE but the fp8
            # V-quantization noise on peaked attention rows sums to ~2.5-4%
            # across the 8 cores' partials -- over the tolerance.)
            va_sb = [
                qkvp.tile([P, KT * 65], f16, tag=f"va{b}", name=f"va{b}")
                for b in range(NB)
            ]
            vb_sb = [
                qkvp.tile([P, KT * 65], f16, tag=f"vb{b}", name=f"vb{b}")
                for b in range(NB)
            ]
            ones_col = ones16[:, 0:KT].rearrange("p (k one) -> p k one", one=1)
            for b in range(NB):
                nc.vector.tensor_copy(
                    va_sb[b].rearrange("p (k c) -> p k c", c=65)[:, :, 64:65],
                    ones_col,
                )
                nc.vector.tensor_copy(
                    vb_sb[b].rearrange("p (k c) -> p k c", c=65)[:, :, 64:65],
                    ones_col,
                )

            _proj_ps = {}

            def proj_half(tch, w_sb, b_sb, dst, half):
                # half 0: open PSUM accumulation, 4 contraction chunks;
                # half 1: 4 more chunks, close group, evict (bias+cast f16)
                def th():
                    if half == 0:
                        ps = psF.tile([P, 512], f32, tag="fx", name="pj")
                        _proj_ps[(tch, id(w_sb))] = ps
                        for c in range(4):
                            nc.tensor.matmul(
                                ps, w_sb[:, c], xtiles[tch][:, c],
                                start=(c == 0), stop=False,
                            )
                    else:
                        ps = _proj_ps.pop((tch, id(w_sb)))
                        for c in range(4, DC):
                            nc.tensor.matmul(
                                ps, w_sb[:, c], xtiles[tch][:, c],
                                start=False, stop=(c == DC - 1),
                            )
                        nc.vector.tensor_scalar_add(dst, ps, b_sb)
                return th

            def K(tch, half):
                b = tch // 4
                dst = kt_sb[b][:, ds((tch % 4) * 512, 512)]
                return proj_half(tch, wk_sb, bk_sb, dst, half)

            def Q(tch, half):
                b = tch // 4
                dst = qt_sb[b][:, ds((tch % 4) * 512, 512)]
                return proj_half(tch, wq_sb, bq_sb, dst, half)

            _vts = {}

            def V(tch, half):
                def th():
                    if half == 0:
                        _vts[tch] = vtsp.tile([P, 512], f32, tag="vts",
                                              name=f"v{tch}")
                    proj_half(tch, wv_sb, bv_sb, _vts[tch], half)()
                return th

            def Vt(tch, half):
                # transpose VT -> V natural; 2 t-tiles of 128 per half.
                # Both transposes into one PSUM tile back-to-back, then the
                # DVE copies drain it (keeps the PE stream dense).
                def th():
                    b = tch // 4
                    vts = _vts[tch]
                    tts = (0, 1) if half == 0 else (2, 3)
                    pvt = psF.tile([P, 256], f32, tag="fx", name="pvt")
                    for j, tt in enumerate(tts):
                        nc.tensor.transpose(
                            pvt[:, ds(j * P, P)], vts[:, ds(tt * P, P)],
                            ident_f,
                        )
                    for j, tt in enumerate(tts):
                        ktile = (tch % 4) * 4 + tt
                        nc.vector.tensor_copy(
                            va_sb[b][:, ds(ktile * 65, DK)],
                            pvt[:, ds(j * P, DK)],
                        )
                        nc.vector.tensor_copy(
                            vb_sb[b][:, ds(ktile * 65, DK)],
                            pvt[:, ds(j * P + DK, DK)],
                        )
                return th

            # ---- the 8 attention chunks as one flat 128-slot pipeline ----
            CHUNKS = [(b, qch) for b in range(NB) for qch in range(4)]
            cstate = [dict() for _ in CHUNKS]

            def fin_thunks(ci):
                # finalize chunk ci: lazy thunks reading cstate[ci], which is
                # populated at the chunk boundary (ua/ub/den0 staged there).
                st = cstate[ci]
                b, qch = CHUNKS[ci]

                def R():
                    # den0[0, 0:512|512:1024] = softmax denominators of both
                    # heads, DMA-staged to partition 0 at the boundary
                    rf = smallp.tile([P, 1024], f32, tag="rf", name="rf")
                    nc.vector.reciprocal_approx_fast(
                        out=rf[0:1, :], in_=st["den0"][0:1, :])
                    rr = smallp.tile([P, 1024], f16, tag="rr", name="rr")
                    nc.vector.tensor_copy(rr[0:1, :], rf[0:1, :])
                    st["rr"] = rr
                    if debug and ci == 0:
                        nc.sync.dma_start(dbg["rr"], rr[0:1, :])

                def bc2():
                    for which in range(2):
                        ps = psF.tile([P, 512], f32, tag="fx", name="bc")
                        nc.tensor.matmul(
                            ps[0:DK, :], ones16[0:1, 0:DK],
                            st["rr"][0:1, ds(which * 512, 512)],
                            start=True, stop=True,
                        )
                        st[f"bc{which}"] = ps

                def mul2():
                    ctq = ctqp.tile([P, 512], f16, tag="ctq", name="ctq")
                    nc.vector.tensor_mul(
                        ctq[0:DK, :], st["ua"][0:DK, :], st["bc0"][0:DK, :])
                    st["ctq"] = ctq
                    tmpb = bsbp.tile([DK, 512], f16, tag="tmpb", name="tmpb")
                    nc.vector.tensor_mul(
                        tmpb, st["ub"][0:DK, :], st["bc1"][0:DK, :])
                    nc.sync.dma_start(st["ctq"][DK:P, :], tmpb)

                def op(tt):
                    def th():
                        ctq = st["ctq"]
                        if debug and ci == 0 and tt == 0:
                            nc.sync.dma_start(dbg["ctq"], ctq)
                        po_sb = posbp.tile([P, 1024], f16, tag="po",
                                           name="po_sb")
                        if ci == len(CHUNKS) - 1:
                            # tail: scores PSUM banks are free; deeper
                            # rotation lets all 8 out-proj matmuls run
                            # back-to-back ahead of the evictions
                            po2 = psS.tile([P, 1024], f32, tag="sc",
                                           name="po2")
                            pos = [po2[:, 0:512], po2[:, 512:1024]]
                        else:
                            pos = []
                        for half in range(2):
                            if ci == len(CHUNKS) - 1:
                                po = pos[half]
                            else:
                                po = psF.tile([P, 512], f32, tag="fx",
                                              name="po")
                                pos.append(po)
                            nc.tensor.matmul(
                                po, ctq[:, ds(tt * P, P)],
                                wo_sb[:, ds(half * 512, 512)],
                                start=True, stop=True,
                            )
                        nc.vector.tensor_copy(po_sb[:, 0:512], pos[0])
                        if ci == len(CHUNKS) - 1:
                            nc.scalar.copy(po_sb[:, 512:1024], pos[1])
                        else:
                            nc.vector.tensor_copy(po_sb[:, 512:1024], pos[1])
                        r0 = b * T + qch * 512 + tt * P
                        nc.sync.dma_start(out[r0: r0 + P, :], po_sb)
                    return th

                return [R, bc2, mul2, op(0), op(1), op(2), op(3)]

            def ctx_mm(ci, kt):
                b, qch = CHUNKS[ci]
                st = cstate[ci]
                e = st["escs"].pop(kt)
                nc.tensor.matmul(
                    st["cxa"], va_sb[b][:, ds(kt * 65, 65)], e[:, 0:512],
                    start=(kt == 0), stop=(kt == KT - 1),
                )
                nc.tensor.matmul(
                    st["cxb"], vb_sb[b][:, ds(kt * 65, 65)], e[:, 512:1024],
                    start=(kt == 0), stop=(kt == KT - 1),
                )

            def boundary(ci):
                # close chunk ci: last ctx, evict accumulators, stage the
                # denominator rows (partition 64) to partition 0 via DMA
                st = cstate[ci]
                ctx_mm(ci, KT - 1)
                ua = bsbp.tile([65, 512], f32, tag="ua", name="ua")
                nc.vector.tensor_copy(ua, st["cxa"])
                den0 = smallp.tile([1, 1024], f32, tag="den0", name="den0")
                nc.sync.dma_start(den0[0:1, 0:512], ua[64:65, :])
                ub = bsbp.tile([65, 512], f32, tag="ub", name="ub")
                nc.vector.tensor_copy(ub, st["cxb"])
                st["ua"], st["ub"] = ua, ub
                nc.sync.dma_start(den0[0:1, 512:1024], ub[64:65, :])
                st["den0"] = den0
                if debug and ci == 0:
                    nc.sync.dma_start(dbg["ua"], ua)
                    nc.sync.dma_start(dbg["ub"], ub)

            sc_ready = {}

            def emit_scores(ci, kt):
                # scores run one slot ahead of their exp so the scalar
                # engine's sem wait is always pre-satisfied
                b, qch = CHUNKS[ci]
                q0 = qch * 512
                sc = psS.tile([P, 1024], f32, tag="sc", name="sc")
                nc.tensor.matmul(
                    sc[:, 0:512],
                    kt_sb[b][0:DK, ds(kt * P, P)],
                    qt_sb[b][0:DK, ds(q0, 512)],
                    start=True, stop=True,
                )
                nc.tensor.matmul(
                    sc[:, 512:1024],
                    kt_sb[b][DK:P, ds(kt * P, P)],
                    qt_sb[b][DK:P, ds(q0, 512)],
                    start=True, stop=True,
                    tile_position=(64, 0),
                )
                sc_ready[(ci, kt)] = sc

            def run_chunk(ci, drains):
                b, qch = CHUNKS[ci]
                st = cstate[ci]
                st["cxa"] = psC.tile([65, 512], f32, tag="cx", name="cxa")
                st["cxb"] = psC.tile([65, 512], f32, tag="cx", name="cxb")
                st["escs"] = {}
                for kt in range(KT):
                    sc = sc_ready.pop((ci, kt))
                    esc = escp.tile([P, 1024], f16, tag="esc", name="esc")
                    nc.scalar.activation(esc, sc, Exp, scale=0.125)
                    st["escs"][kt] = esc
                    if kt < KT - 1:
                        emit_scores(ci, kt + 1)
                    elif ci + 1 < len(CHUNKS):
                        emit_scores(ci + 1, 0)
                    if kt == 0:
                        if ci > 0:
                            boundary(ci - 1)
                    else:
                        ctx_mm(ci, kt - 1)
                    for th in drains[kt]:
                        th()

            def sched(*slots):
                d = [[] for _ in range(KT)]
                for i, s in enumerate(slots):
                    if s:
                        d[i] = list(s) if isinstance(s, (list, tuple)) else [s]
                return d

            # brief HAM warmup, then K0 emitted as a single 8-chunk
            # accumulation: each matmul depends only on its own x0 quarter,
            # so the first one starts as soon as 128KB of x has landed
            for w in range(3):
                wt = psS.tile([P, 1024], f32, tag="sc", name="warm")
                nc.tensor.matmul(wt[:, 0:512], ident, ones16,
                                 start=True, stop=True)
            ps0 = psF.tile([P, 512], f32, tag="fx", name="pj0")
            for c in range(DC):
                nc.tensor.matmul(ps0, wk_sb[:, c], x0[:, c],
                                 start=(c == 0), stop=(c == DC - 1))
            nc.vector.tensor_scalar_add(kt_sb[0][:, 0:512], ps0, bk_sb)
            Q(0, 0)(); Q(0, 1)()
            V(0, 0)(); V(0, 1)()
            Vt(0, 0)(); Vt(0, 1)()
            emit_scores(0, 0)

            # chunk 0: remaining b0 projections (x1 load issued in head
            # epilogue below; x2/x3/wo issues spread through the chunk)
            run_chunk(0, sched(
                [K(1, 0), K(1, 1)], [V(1, 0), V(1, 1)], [Vt(1, 0), Vt(1, 1)],
                [load_x(3), K(2, 0), K(2, 1)], [V(2, 0), V(2, 1)],
                [Vt(2, 0), Vt(2, 1)],
                K(3, 0), K(3, 1), V(3, 0), V(3, 1), [load_wo, Vt(3, 0)],
                Vt(3, 1), Q(1, 0), Q(1, 1), None, None,
            ))
            f = fin_thunks(0)
            run_chunk(1, sched(
                [load_x(4), f[0]], f[1], f[2], Q(2, 0), Q(2, 1), f[3],
                K(4, 0), K(4, 1), f[4], [load_x(5), V(4, 0)], V(4, 1),
                f[5], f[6], [Vt(4, 0), Vt(4, 1)], None, None,
            ))
            f = fin_thunks(1)
            run_chunk(2, sched(
                [Q(3, 0), f[0]], Q(3, 1), f[1], f[2], Q(4, 0), Q(4, 1),
                f[3], [load_x(6), K(5, 0)], K(5, 1), f[4], V(5, 0), V(5, 1),
                f[5], f[6], [Vt(5, 0), Vt(5, 1)], None,
            ))
            f = fin_thunks(2)
            run_chunk(3, sched(
                f[0], f[1], f[2], [load_x(7), K(6, 0)], K(6, 1), f[3],
                V(6, 0), V(6, 1), f[4], [Vt(6, 0), Vt(6, 1)],
                K(7, 0), K(7, 1), f[5], V(7, 0), V(7, 1), f[6],
            ))
            f = fin_thunks(3)
            run_chunk(4, sched(
                [Vt(7, 0), Vt(7, 1), f[0]], f[1], f[2], Q(5, 0), Q(5, 1),
                f[3], f[4], f[5], f[6], None, None, None, None, None,
                None, None,
            ))
            f = fin_thunks(4)
            run_chunk(5, sched(
                f[0], f[1], f[2], Q(6, 0), Q(6, 1), f[3], f[4], f[5], f[6],
                None, None, None, None, None, None, None,
            ))
            f = fin_thunks(5)
            run_chunk(6, sched(
                f[0], f[1], f[2], Q(7, 0), Q(7, 1), f[3], f[4], f[5], f[6],
                None, None, None, None, None, None, None,
            ))
            f = fin_thunks(6)
            run_chunk(7, sched(
                f[0], f[1], f[2], f[3], f[4], f[5], f[6],
                None, None, None, None, None, None, None, None, None,
            ))
            # tail: close and finalize the last chunk
            boundary(7)
            for th in fin_thunks(7):
                th()
            if debug:
                nc.sync.dma_start(dbg["qt"], qt_sb[0])
                nc.sync.dma_start(dbg["kt"], kt_sb[0])

    nc.compile()
    return nc


def _get_nc(reps=1, debug=False):
    key = f"nc{reps}_{debug}"
    if key not in _CACHE:
        _CACHE[key] = _build(reps, debug=debug)
    return _CACHE[key]


def kernel(x, Wq, bq, Wk, bk, Wv, bv, Wo, bo):
    from concourse.bass_utils import run_bass_kernel_spmd

    x = np.asarray(x, dtype=np.float32)
    Wq = np.asarray(Wq, dtype=np.float32)
    Wk = np.asarray(Wk, dtype=np.float32)
    Wv = np.asarray(Wv, dtype=np.float32)
    Wo = np.asarray(Wo, dtype=np.float32)
    bq = np.asarray(bq, dtype=np.float32)
    bk = np.asarray(bk, dtype=np.float32)
    bv = np.asarray(bv, dtype=np.float32)
    bo = np.asarray(bo, dtype=np.float32)

    B, Tl, Dl = x.shape
    xt = np.ascontiguousarray(x.reshape(B * Tl, Dl).T.astype(np.float16))

    in_maps = []
    for c in range(NCORES):
        sl = slice(c * P, (c + 1) * P)
        in_maps.append(
            {
                "xt": xt,
                "wq": np.ascontiguousarray(Wq[sl, :].T.astype(np.float16)),
                "wk": np.ascontiguousarray(Wk[sl, :].T.astype(np.float16)),
                "wv": np.ascontiguousarray(Wv[sl, :].T.astype(np.float16)),
                "wo": np.ascontiguousarray(Wo[:, sl].T.astype(np.float16)),
                "bq": np.ascontiguousarray(bq[sl].reshape(P, 1)),
                "bk": np.ascontiguousarray(bk[sl].reshape(P, 1)),
                "bv": np.ascontiguousarray(bv[sl].reshape(P, 1)),
            }
        )

    nc = _get_nc()
    _CACHE["in_maps"] = in_maps
    res = run_bass_kernel_spmd(nc, in_maps, core_ids=list(range(NCORES)))
    acc = res.results[0]["out"].astype(np.float32)
    for c in range(1, NCORES):
        acc = acc + res.results[c]["out"].astype(np.float32)
    acc = acc + bo[None, :]
    return acc.reshape(B, Tl, Dl).astype(np.float32)
